# revision 15
# baseline (speedup 1.0000x reference)
"""Trainium2 Bass kernel for nn_ConvSelfAttention.

Math: the reference computes, per head h,
    kv   = conv3x3(x, w_kv[h]) + b_kv[h]                     # [B*T,19,19,16]
    q    = conv3x3(x, w_q[h])  + b_q[h]
    att[b,tq,tk] = conv3x3(concat[kv[tk], q[tq]], w_att[h]) + b_att[h]
                 = A_k[b,tk] + A_q[b,tq] + b_att[h]          # conv is linear
    soft = softmax_tk(att) = softmax_tk(A_k[b,tk])           # tq-terms cancel
    out[b,tq] = sum_tk kv[b,tk] * soft[b,tk]                 # independent of tq
So the q path (w_q, b_q) and b_att never affect the output, and the result
broadcasts over the query-time axis.

Device work per core (8 cores = 4 batches x 2 head-pairs, fully independent):
    stage A: kv conv; K=128 matmuls cover 2 taps at once via a host-sent
             copy of x shifted by one pixel in partitions 64-127
             (3 tap-pairs + 3 single taps = 6 matmuls/img instead of 9)
    stage B: score conv; one K=128 matmul per (img-group, tap) computes the
             scalar scores of 4 imgs x 2 heads via block-diagonal weights,
             all 72 matmuls accumulate into a single [64, 361] PSUM tile
    transpose s and kv to pixel-major via PE transposes
    softmax over key-time + weighted sum on DVE with broadcast APs.
"""

import sys

import ml_dtypes
import numpy as np

if "/opt/trn_rl_repo" not in sys.path:
    sys.path.insert(0, "/opt/trn_rl_repo")

import concourse.bass as bass
import concourse.mybir as mybir
import concourse.tile as tile
from concourse import bacc
from concourse.bass_utils import run_bass_kernel_spmd

# problem constants (hardcoded per contract)
B, T, HS, WS, C, NH = 4, 32, 19, 19, 64, 4
D = C // NH            # 16 per-head channels
PX = HS * WS           # 361 pixels
NCORE = 8
HPC = 2                # heads per core
M32 = HPC * D          # 32 kv channels per core
NIMG = T * 441         # flat padded-image columns
CHUNKS = [(0, 128), (128, 128), (256, 105)]  # pixel chunks (start, count)

# stage A: one K=128 matmul per tap computes 2 images at once via
# block-diagonal weights: partitions 0-63 = half-0 image, 64-127 = half-1

F32 = mybir.dt.float32
BF16 = mybir.dt.bfloat16
MMDT = BF16


def _kernel_body(tc, y, x_t, x_t2, w_kv_t, w_s_t, b_vec, ident):
    nc = tc.nc

    from contextlib import ExitStack

    with ExitStack() as ctx:
        const = ctx.enter_context(tc.tile_pool(name="const", bufs=1))
        kvpool = ctx.enter_context(tc.tile_pool(name="kv", bufs=1))
        sbig = ctx.enter_context(tc.tile_pool(name="sbig", bufs=1))
        small = ctx.enter_context(tc.tile_pool(name="small", bufs=1))
        tmppool = ctx.enter_context(tc.tile_pool(name="tmp", bufs=2))
        psA = ctx.enter_context(tc.tile_pool(name="psA", bufs=3, space="PSUM"))
        psS = ctx.enter_context(tc.tile_pool(name="psS", bufs=1, space="PSUM"))
        psT = ctx.enter_context(tc.tile_pool(name="psT", bufs=2, space="PSUM"))
        # ---- load inputs -------------------------------------------------
        # x first (stage A is the critical path): small first chunk so the
        # first matmuls start ASAP, then the kv weights, then the rest
        HIMG = 16 * 441
        x_sb = const.tile([128, HIMG], MMDT)
        w_kv_sb = const.tile([128, 9 * 2 * M32], MMDT)
        sl0 = slice(0, 2 * 441)
        nc.sync.dma_start(x_sb[0:64, sl0], x_t[:, sl0])
        nc.sync.dma_start(x_sb[64:128, sl0], x_t2[:, sl0])
        nc.sync.dma_start(w_kv_sb[:], w_kv_t[:])
        bounds = [i * 441 for i in range(2, 17, 2)]
        for c0, c1 in zip(bounds[:-1], bounds[1:]):
            sl = slice(c0, c1)
            nc.sync.dma_start(x_sb[0:64, sl], x_t[:, sl])
            nc.sync.dma_start(x_sb[64:128, sl], x_t2[:, sl])
        w_s_sb = const.tile([128, 4 * 9 * M32], MMDT)
        nc.sync.dma_start(w_s_sb[:], w_s_t[:])
        b_sb = const.tile([128, 1], F32)
        nc.sync.dma_start(b_sb[:], b_vec[:])
        id_sb = const.tile([128, 128], F32)
        nc.sync.dma_start(id_sb[:], ident[:])

        # ---- stage A: kv conv; stage B: score conv -----------------------
        kv = [kvpool.tile([128, PX], F32, tag=f"kv{g}", name=f"kv{g}")
              for g in range(8)]
        # padded (21x21) bf16 kv for stage-B windowed rhs reads
        kvb = [kvpool.tile([128, 441], BF16, tag=f"kvb{g}", name=f"kvb{g}")
               for g in range(8)]
        for g in range(8):  # zero the 1-px borders once
            v = kvb[g][:].rearrange("p (a b) -> p a b", a=21)
            nc.gpsimd.memset(v[:, 0:21:20, :], 0.0)
            nc.gpsimd.memset(v[:, 1:20, 0:21:20], 0.0)
        # scores for all 32 imgs x 2 heads accumulate into one PSUM tile;
        # partition = 32*half + 2*i_half + h  (i_half = img index in half)
        s_all = psS.tile([64, PX], F32, tag="sall", name="sall")
        # kv -> pixel-major transposes, issued per half as soon as that
        # half's kv tiles are evacuated (overlaps with stage A of next half)
        kvT = [sbig.tile([128, 1024], BF16, tag=f"kvT{c}", name=f"kvT{c}")
               for c in range(3)]

        def stage_b(g):
            # one K=128 matmul per tap scores all 4 imgs x 2 heads of tile g
            half, q = g // 4, g % 4
            kvv = kvb[g][:].rearrange("p (a b) -> p a b", a=21)
            for tap in range(9):
                dy, dx = tap // 3, tap % 3
                nc.tensor.matmul(
                    s_all[32 * half : 32 * half + 32, :],
                    w_s_sb[:, (q * 9 + tap) * M32 : (q * 9 + tap + 1) * M32],
                    kvv[:, dy : dy + HS, dx : dx + WS],
                    start=(q == 0 and tap == 0),
                    stop=(q == 3 and tap == 8),
                    tile_position=(0, 32 * half),
                    skip_group_check=True,
                )

        def kvt_transpose(half):
            for c, (p0, cnt) in enumerate(CHUNKS):
                ps_k = psT.tile([128, 512], F32, tag="psTk", name=f"psTk{half}{c}")
                for gi in range(4):
                    g = half * 4 + gi
                    nc.tensor.matmul(
                        ps_k[0:cnt, gi * 128 : (gi + 1) * 128],
                        kv[g][:, p0 : p0 + cnt], id_sb,
                        is_transpose=True,
                        start=(gi == 0), stop=(gi == 3),
                        skip_group_check=True,
                    )
                nc.scalar.copy(kvT[c][0:cnt, half * 512 : (half + 1) * 512],
                               ps_k[0:cnt, :])

        for g in range(8):
            ps_g = psA.tile([128, PX], F32, tag="psA", name=f"psA{g}")
            for jj in range(2):
                i = 4 * (g // 2) + 2 * (g % 2) + jj
                xview = x_sb[:, i * 441 : (i + 1) * 441].rearrange(
                    "p (a b) -> p a b", a=21)
                for tap in range(9):
                    dy, dx = tap // 3, tap % 3
                    nc.tensor.matmul(
                        ps_g[64 * jj : 64 * jj + 64, :],
                        w_kv_sb[:, tap * 64 : tap * 64 + 64],
                        xview[:, dy : dy + HS, dx : dx + WS],
                        start=(tap == 0), stop=(tap == 8),
                        tile_position=(0, 64 * jj),
                        skip_group_check=True,
                    )
            # evacuate: biased fp32 kv (DVE) and biasless bf16 kvb (Act) both
            # read the PSUM tile directly so they run in parallel; the bias
            # shifts every key image's score equally per pixel, so it cancels
            # in the softmax and kvb can skip it
            nc.vector.tensor_scalar_add(kv[g][:], ps_g[:], b_sb[:])
            kvb_in = kvb[g][:].rearrange("p (a b) -> p a b", a=21)[:, 1:20, 1:20]
            nc.scalar.copy(kvb_in, ps_g[:].rearrange("p (a b) -> p a b", a=HS))
            # stage B for the previous group overlaps this group's evacuation
            if g >= 1:
                stage_b(g - 1)
            if g == 4:
                kvt_transpose(0)
        kvt_transpose(1)
        stage_b(7)

        # ---- scores -> pixel-major, exp ---------------------------------
        s_sb = sbig.tile([64, PX], F32)
        nc.scalar.copy(s_sb[:], s_all[0:64, :])
        p_T = sbig.tile([128, 192], BF16)  # exp(s), cols = 64*c + 32*hf+2*i+h
        for c, (p0, cnt) in enumerate(CHUNKS):
            ps_t = psT.tile([128, 64], F32, tag="psTs", name="psTs")
            nc.tensor.matmul(ps_t[0:cnt, :], s_sb[:, p0 : p0 + cnt],
                             id_sb[0:64, 0:64], is_transpose=True)
            nc.scalar.activation(p_T[0:cnt, 64 * c : 64 * c + 64], ps_t[0:cnt, :],
                                 mybir.ActivationFunctionType.Exp)

        # ---- softmax normalizer + weighted sum --------------------------
        for c, (p0, cnt) in enumerate(CHUNKS):
            z2 = small.tile([128, 2], F32, tag=f"z2{c}", name=f"z2{c}")
            nc.vector.reduce_sum(
                z2[0:cnt, :],
                p_T[0:cnt, 64 * c : 64 * c + 64].rearrange(
                    "p (a h) -> p h a", a=32, h=2),
                axis=mybir.AxisListType.X,
            )
            zi = small.tile([128, 2], F32, tag=f"zi{c}", name=f"zi{c}")
            nc.vector.reciprocal(zi[0:cnt, :], z2[0:cnt, :])

            outT = small.tile([128, M32], F32, tag=f"outT{c}", name=f"outT{c}")
            for h in range(HPC):
                # tmp[p, d, hf, i] = kvT[p, img(hf,i), 16h+d] * p_T[p, hf, i]
                v0 = kvT[c][0:cnt, :].rearrange(
                    "p (tp jj hf h d) -> p h d tp jj hf",
                    tp=8, jj=2, hf=2, h=2)[:, h]
                pv = p_T[0:cnt, 64 * c : 64 * c + 64].rearrange(
                    "p (tp jj hf h) -> p h tp jj hf",
                    tp=8, jj=2, hf=2, h=2)[:, h]
                v1 = bass.AP(tensor=pv.tensor, offset=pv.offset,
                             ap=[pv.ap[0], [0, D], pv.ap[1], pv.ap[2],
                                 pv.ap[3]])
                meng = nc.vector if h == 0 else nc.gpsimd
                t = tmppool.tile([128, 512], BF16, name=f"tmp{c}{h}")
                meng.tensor_mul(t[0:cnt, :], v0, v1)
                acc = small.tile([128, D], F32, tag=f"acc{c}{h}", name=f"acc{c}{h}")
                nc.vector.reduce_sum(
                    acc[0:cnt, :],
                    t[0:cnt, :].rearrange("p (d r) -> p d r", d=D),
                    axis=mybir.AxisListType.X,
                )
                nc.vector.tensor_scalar_mul(
                    outT[0:cnt, D * h : D * h + D], acc[0:cnt, :],
                    zi[0:cnt, h : h + 1],
                )
            nc.sync.dma_start(y[p0 : p0 + cnt, :], outT[0:cnt, :])


_CACHE = {}


def _build_program():
    if "nc" in _CACHE:
        return _CACHE["nc"]
    nc = bacc.Bacc("TRN2", target_bir_lowering=False, debug=False,
                   num_devices=NCORE)
    x_t = nc.dram_tensor("x_t", [C, 16 * 441], MMDT, kind="ExternalInput").ap()
    x_t2 = nc.dram_tensor("x_t2", [C, 16 * 441], MMDT,
                          kind="ExternalInput").ap()
    w_kv_t = nc.dram_tensor("w_kv_t", [128, 9 * 2 * M32], MMDT,
                            kind="ExternalInput").ap()
    w_s_t = nc.dram_tensor("w_s_t", [128, 4 * 9 * M32], MMDT,
                           kind="ExternalInput").ap()
    b_vec = nc.dram_tensor("b_vec", [128, 1], F32, kind="ExternalInput").ap()
    ident = nc.dram_tensor("ident", [128, 128], F32, kind="ExternalInput").ap()
    y = nc.dram_tensor("y", [PX, M32], F32, kind="ExternalOutput").ap()
    with tile.TileContext(nc) as tc:
        _kernel_body(tc, y, x_t, x_t2, w_kv_t, w_s_t, b_vec, ident)
    nc.compile()
    _CACHE["nc"] = nc
    return nc


def make_in_maps(x, w_kv, b_kv, w_att):
    """Host-side shard prep: per-core input dicts."""
    x = np.asarray(x, np.float32)
    w_kv = np.asarray(w_kv, np.float32)
    b_kv = np.asarray(b_kv, np.float32)
    w_att = np.asarray(w_att, np.float32)
    ident = np.eye(128, dtype=np.float32)
    in_maps = []
    # channel-major x per batch: [64, T*441] padded 21x21, plus shift-by-1
    xt_all, xt2_all = [], []
    for b in range(B):
        xp = np.zeros((C, T, 21, 21), np.float32)
        xp[:, :, 1:20, 1:20] = x[b].transpose(3, 0, 1, 2)
        flat = xp.reshape(C, T, 441)
        xt_all.append(flat[:, 0:16].reshape(C, 16 * 441).astype(
            ml_dtypes.bfloat16))
        xt2_all.append(flat[:, 16:32].reshape(C, 16 * 441).astype(
            ml_dtypes.bfloat16))
    for core in range(NCORE):
        b, hb = core // 2, (core % 2) * HPC
        # stage-A weights: block-diagonal per tap; rows 0-63 (half-0 img)
        # feed cols 0-31, rows 64-127 (half-1 img) feed cols 32-63
        wk = np.zeros((128, 9 * 2 * M32), np.float32)
        for tap in range(9):
            dy, dx = tap // 3, tap % 3
            for h in range(HPC):
                wk[0:64, 64 * tap + D * h : 64 * tap + D * (h + 1)] = \
                    w_kv[hb + h, dy, dx]
                wk[64:128, 64 * tap + 32 + D * h : 64 * tap + 32 + D * (h + 1)] = \
                    w_kv[hb + h, dy, dx]
        # stage-B weights: block (q, tap) maps img j of group (row 32j +
        # 16h + d) to score column 8q + 2j + h
        ws = np.zeros((128, 4, 9, M32), np.float32)
        for qq in range(4):
            for tap in range(9):
                dy, dx = tap // 3, tap % 3
                for j in range(4):
                    for h in range(HPC):
                        ws[32 * j + D * h : 32 * j + D * (h + 1), qq, tap,
                           8 * qq + 2 * j + h] = w_att[hb + h, dy, dx, :D, 0]
        ws = ws.reshape(128, 4 * 9 * M32)
        bv = np.zeros((128, 1), np.float32)
        bv[:, 0] = np.tile(np.concatenate([b_kv[hb], b_kv[hb + 1]]), 4)
        in_maps.append({"x_t": xt_all[b], "x_t2": xt2_all[b],
                        "w_kv_t": wk.astype(ml_dtypes.bfloat16),
                        "w_s_t": ws.astype(ml_dtypes.bfloat16),
                        "b_vec": bv, "ident": ident})
    return in_maps


def assemble(results):
    out = np.empty((B, T, HS, WS, C), np.float32)
    for core in range(NCORE):
        b, hb = core // 2, (core % 2) * M32
        yc = np.asarray(results[core]["y"]).reshape(HS, WS, M32)
        out[b, :, :, :, hb : hb + M32] = yc[None]
    return out


def kernel(x, w_q, b_q, w_kv, b_kv, w_att, b_att, **_unused):
    nc = _build_program()
    in_maps = make_in_maps(x, w_kv, b_kv, w_att)
    res = run_bass_kernel_spmd(nc, in_maps, core_ids=list(range(NCORE)))
    return assemble(res.results)


if __name__ == "__main__":
    rng = np.random.default_rng(0)
    ins = {
        "x": rng.standard_normal((B, T, HS, WS, C)).astype(np.float32),
        "w_q": rng.standard_normal((NH, 3, 3, C, D)).astype(np.float32) * 0.05,
        "b_q": np.zeros((NH, D), np.float32),
        "w_kv": rng.standard_normal((NH, 3, 3, C, D)).astype(np.float32) * 0.05,
        "b_kv": np.zeros((NH, D), np.float32),
        "w_att": rng.standard_normal((NH, 3, 3, 2 * D, 1)).astype(np.float32) * 0.05,
        "b_att": np.zeros((NH, 1), np.float32),
    }
    out = kernel(**ins)
    print("kernel output", out.shape, out.dtype)


# revision 16
# speedup vs baseline: 1.0075x; 1.0075x over previous
"""Trainium2 Bass kernel for nn_ConvSelfAttention.

Math: the reference computes, per head h,
    kv   = conv3x3(x, w_kv[h]) + b_kv[h]                     # [B*T,19,19,16]
    q    = conv3x3(x, w_q[h])  + b_q[h]
    att[b,tq,tk] = conv3x3(concat[kv[tk], q[tq]], w_att[h]) + b_att[h]
                 = A_k[b,tk] + A_q[b,tq] + b_att[h]          # conv is linear
    soft = softmax_tk(att) = softmax_tk(A_k[b,tk])           # tq-terms cancel
    out[b,tq] = sum_tk kv[b,tk] * soft[b,tk]                 # independent of tq
So the q path (w_q, b_q) and b_att never affect the output, and the result
broadcasts over the query-time axis.

Device work per core (8 cores = 4 batches x 2 head-pairs, fully independent):
    stage A: kv conv; one K=128 matmul per tap computes 2 images at once
             (partitions 0-63 = a half-0 image, 64-127 = a half-1 image,
             block-diagonal weights) -> 4.5 matmuls/img instead of 9
    stage B: score conv; one K=128 matmul per (img-tile, tap) computes the
             scalar scores of 4 imgs x 2 heads via block-diagonal weights,
             all 72 matmuls accumulate into a single [64, 361] PSUM tile
    stage B for tile g-1 is issued during stage A of tile g (software
    pipelining) so PE never waits on the PSUM evacuation engines
    transpose s and kv to pixel-major via PE transposes
    softmax over key-time + weighted sum on DVE with broadcast APs.
"""

import sys

import ml_dtypes
import numpy as np

if "/opt/trn_rl_repo" not in sys.path:
    sys.path.insert(0, "/opt/trn_rl_repo")

import concourse.bass as bass
import concourse.mybir as mybir
import concourse.tile as tile
from concourse import bacc
from concourse.bass_utils import run_bass_kernel_spmd

# problem constants (hardcoded per contract)
B, T, HS, WS, C, NH = 4, 32, 19, 19, 64, 4
D = C // NH            # 16 per-head channels
PX = HS * WS           # 361 pixels
NCORE = 8
HPC = 2                # heads per core
M32 = HPC * D          # 32 kv channels per core
NIMG = T * 441         # flat padded-image columns
CHUNKS = [(0, 128), (128, 128), (256, 105)]  # pixel chunks (start, count)

# stage A: one K=128 matmul per tap computes 2 images at once via
# block-diagonal weights: partitions 0-63 = half-0 image, 64-127 = half-1

F32 = mybir.dt.float32
BF16 = mybir.dt.bfloat16
MMDT = BF16


def _kernel_body(tc, y, x_t, x_t2, w_kv_t, w_s_t, b_vec, ident):
    nc = tc.nc

    from contextlib import ExitStack

    with ExitStack() as ctx:
        const = ctx.enter_context(tc.tile_pool(name="const", bufs=1))
        kvpool = ctx.enter_context(tc.tile_pool(name="kv", bufs=1))
        sbig = ctx.enter_context(tc.tile_pool(name="sbig", bufs=1))
        small = ctx.enter_context(tc.tile_pool(name="small", bufs=1))
        tmppool = ctx.enter_context(tc.tile_pool(name="tmp", bufs=2))
        psA = ctx.enter_context(tc.tile_pool(name="psA", bufs=3, space="PSUM"))
        psS = ctx.enter_context(tc.tile_pool(name="psS", bufs=1, space="PSUM"))
        psT = ctx.enter_context(tc.tile_pool(name="psT", bufs=2, space="PSUM"))
        # ---- load inputs -------------------------------------------------
        # x first (stage A is the critical path): small first chunk so the
        # first matmuls start ASAP, then the kv weights, then the rest
        HIMG = 16 * 441
        x_sb = const.tile([128, HIMG], MMDT)
        w_kv_sb = const.tile([128, 9 * 2 * M32], MMDT)
        sl0 = slice(0, 2 * 441)
        nc.sync.dma_start(x_sb[0:64, sl0], x_t[:, sl0])
        nc.sync.dma_start(x_sb[64:128, sl0], x_t2[:, sl0])
        nc.sync.dma_start(w_kv_sb[:], w_kv_t[:])
        bounds = [i * 441 for i in range(2, 17, 2)]
        for c0, c1 in zip(bounds[:-1], bounds[1:]):
            sl = slice(c0, c1)
            nc.sync.dma_start(x_sb[0:64, sl], x_t[:, sl])
            nc.sync.dma_start(x_sb[64:128, sl], x_t2[:, sl])
        w_s_sb = const.tile([128, 4 * 9 * M32], MMDT)
        nc.sync.dma_start(w_s_sb[:], w_s_t[:])
        b_sb = const.tile([128, 1], F32)
        nc.sync.dma_start(b_sb[:], b_vec[:])
        id_sb = const.tile([128, 128], F32)
        nc.sync.dma_start(id_sb[:], ident[:])

        # ---- stage A: kv conv; stage B: score conv -----------------------
        kv = [kvpool.tile([128, PX], F32, tag=f"kv{g}", name=f"kv{g}")
              for g in range(8)]
        # padded (21x21) bf16 kv for stage-B windowed rhs reads
        kvb = [kvpool.tile([128, 441], BF16, tag=f"kvb{g}", name=f"kvb{g}")
               for g in range(8)]
        for g in range(8):  # zero the 1-px borders once
            v = kvb[g][:].rearrange("p (a b) -> p a b", a=21)
            nc.gpsimd.memset(v[:, 0:21:20, :], 0.0)
            nc.gpsimd.memset(v[:, 1:20, 0:21:20], 0.0)
        # scores for all 32 imgs x 2 heads accumulate into one PSUM tile;
        # partition = 32*half + 2*i_half + h  (i_half = img index in half)
        s_all = psS.tile([64, PX], F32, tag="sall", name="sall")
        # kv -> pixel-major transposes, issued per half as soon as that
        # half's kv tiles are evacuated (overlaps with stage A of next half)
        kvT = [sbig.tile([128, 1024], BF16, tag=f"kvT{c}", name=f"kvT{c}")
               for c in range(3)]

        def stage_b(g):
            # one K=128 matmul per tap scores all 4 imgs x 2 heads of tile g
            half, q = g // 4, g % 4
            kvv = kvb[g][:].rearrange("p (a b) -> p a b", a=21)
            for tap in range(9):
                dy, dx = tap // 3, tap % 3
                nc.tensor.matmul(
                    s_all[32 * half : 32 * half + 32, :],
                    w_s_sb[:, (q * 9 + tap) * M32 : (q * 9 + tap + 1) * M32],
                    kvv[:, dy : dy + HS, dx : dx + WS],
                    start=(q == 0 and tap == 0),
                    stop=(q == 3 and tap == 8),
                    tile_position=(0, 32 * half),
                    skip_group_check=True,
                )

        def kvt_transpose(half):
            for c, (p0, cnt) in enumerate(CHUNKS):
                ps_k = psT.tile([128, 512], F32, tag="psTk", name=f"psTk{half}{c}")
                for gi in range(4):
                    g = half * 4 + gi
                    nc.tensor.matmul(
                        ps_k[0:cnt, gi * 128 : (gi + 1) * 128],
                        kv[g][:, p0 : p0 + cnt], id_sb,
                        is_transpose=True,
                        start=(gi == 0), stop=(gi == 3),
                        skip_group_check=True,
                    )
                nc.scalar.copy(kvT[c][0:cnt, half * 512 : (half + 1) * 512],
                               ps_k[0:cnt, :])

        for g in range(8):
            ps_g = psA.tile([128, PX], F32, tag="psA", name=f"psA{g}")
            for jj in range(2):
                i = 4 * (g // 2) + 2 * (g % 2) + jj
                xview = x_sb[:, i * 441 : (i + 1) * 441].rearrange(
                    "p (a b) -> p a b", a=21)
                for tap in range(9):
                    dy, dx = tap // 3, tap % 3
                    nc.tensor.matmul(
                        ps_g[64 * jj : 64 * jj + 64, :],
                        w_kv_sb[:, tap * 64 : tap * 64 + 64],
                        xview[:, dy : dy + HS, dx : dx + WS],
                        start=(tap == 0), stop=(tap == 8),
                        tile_position=(0, 64 * jj),
                        skip_group_check=True,
                    )
            # evacuate: biased fp32 kv (DVE) and biasless bf16 kvb (Act) both
            # read the PSUM tile directly so they run in parallel; the bias
            # shifts every key image's score equally per pixel, so it cancels
            # in the softmax and kvb can skip it
            nc.vector.tensor_scalar_add(kv[g][:], ps_g[:], b_sb[:])
            kvb_in = kvb[g][:].rearrange("p (a b) -> p a b", a=21)[:, 1:20, 1:20]
            nc.scalar.copy(kvb_in, ps_g[:].rearrange("p (a b) -> p a b", a=HS))
            # stage B for the previous group overlaps this group's evacuation
            if g >= 1:
                stage_b(g - 1)
            if g == 4:
                kvt_transpose(0)
        stage_b(7)
        kvt_transpose(1)

        # ---- scores -> pixel-major, exp ---------------------------------
        s_sb = sbig.tile([64, PX], F32)
        nc.scalar.copy(s_sb[:], s_all[0:64, :])
        p_T = sbig.tile([128, 192], BF16)  # exp(s), cols = 64*c + 32*hf+2*i+h
        for c, (p0, cnt) in enumerate(CHUNKS):
            ps_t = psT.tile([128, 64], F32, tag="psTs", name="psTs")
            nc.tensor.matmul(ps_t[0:cnt, :], s_sb[:, p0 : p0 + cnt],
                             id_sb[0:64, 0:64], is_transpose=True)
            nc.scalar.activation(p_T[0:cnt, 64 * c : 64 * c + 64], ps_t[0:cnt, :],
                                 mybir.ActivationFunctionType.Exp)

        # ---- softmax normalizer + weighted sum --------------------------
        for c, (p0, cnt) in enumerate(CHUNKS):
            z2 = small.tile([128, 2], F32, tag=f"z2{c}", name=f"z2{c}")
            nc.vector.reduce_sum(
                z2[0:cnt, :],
                p_T[0:cnt, 64 * c : 64 * c + 64].rearrange(
                    "p (a h) -> p h a", a=32, h=2),
                axis=mybir.AxisListType.X,
            )
            zi = small.tile([128, 2], F32, tag=f"zi{c}", name=f"zi{c}")
            nc.vector.reciprocal(zi[0:cnt, :], z2[0:cnt, :])

            outT = small.tile([128, M32], F32, tag=f"outT{c}", name=f"outT{c}")
            for h in range(HPC):
                # tmp[p, d, hf, i] = kvT[p, img(hf,i), 16h+d] * p_T[p, hf, i]
                v0 = kvT[c][0:cnt, :].rearrange(
                    "p (tp jj hf h d) -> p h d tp jj hf",
                    tp=8, jj=2, hf=2, h=2)[:, h]
                pv = p_T[0:cnt, 64 * c : 64 * c + 64].rearrange(
                    "p (tp jj hf h) -> p h tp jj hf",
                    tp=8, jj=2, hf=2, h=2)[:, h]
                v1 = bass.AP(tensor=pv.tensor, offset=pv.offset,
                             ap=[pv.ap[0], [0, D], pv.ap[1], pv.ap[2],
                                 pv.ap[3]])
                meng = nc.vector if h == 0 else nc.gpsimd
                t = tmppool.tile([128, 512], BF16, name=f"tmp{c}{h}")
                meng.tensor_mul(t[0:cnt, :], v0, v1)
                acc = small.tile([128, D], F32, tag=f"acc{c}{h}", name=f"acc{c}{h}")
                nc.vector.reduce_sum(
                    acc[0:cnt, :],
                    t[0:cnt, :].rearrange("p (d r) -> p d r", d=D),
                    axis=mybir.AxisListType.X,
                )
                nc.vector.tensor_scalar_mul(
                    outT[0:cnt, D * h : D * h + D], acc[0:cnt, :],
                    zi[0:cnt, h : h + 1],
                )
            nc.sync.dma_start(y[p0 : p0 + cnt, :], outT[0:cnt, :])


_CACHE = {}


def _build_program():
    if "nc" in _CACHE:
        return _CACHE["nc"]
    nc = bacc.Bacc("TRN2", target_bir_lowering=False, debug=False,
                   num_devices=NCORE)
    x_t = nc.dram_tensor("x_t", [C, 16 * 441], MMDT, kind="ExternalInput").ap()
    x_t2 = nc.dram_tensor("x_t2", [C, 16 * 441], MMDT,
                          kind="ExternalInput").ap()
    w_kv_t = nc.dram_tensor("w_kv_t", [128, 9 * 2 * M32], MMDT,
                            kind="ExternalInput").ap()
    w_s_t = nc.dram_tensor("w_s_t", [128, 4 * 9 * M32], MMDT,
                           kind="ExternalInput").ap()
    b_vec = nc.dram_tensor("b_vec", [128, 1], F32, kind="ExternalInput").ap()
    ident = nc.dram_tensor("ident", [128, 128], F32, kind="ExternalInput").ap()
    y = nc.dram_tensor("y", [PX, M32], F32, kind="ExternalOutput").ap()
    with tile.TileContext(nc) as tc:
        _kernel_body(tc, y, x_t, x_t2, w_kv_t, w_s_t, b_vec, ident)
    nc.compile()
    _CACHE["nc"] = nc
    return nc


def make_in_maps(x, w_kv, b_kv, w_att):
    """Host-side shard prep: per-core input dicts."""
    x = np.asarray(x, np.float32)
    w_kv = np.asarray(w_kv, np.float32)
    b_kv = np.asarray(b_kv, np.float32)
    w_att = np.asarray(w_att, np.float32)
    ident = np.eye(128, dtype=np.float32)
    in_maps = []
    # channel-major x per batch: [64, T*441] padded 21x21, plus shift-by-1
    xt_all, xt2_all = [], []
    for b in range(B):
        xp = np.zeros((C, T, 21, 21), np.float32)
        xp[:, :, 1:20, 1:20] = x[b].transpose(3, 0, 1, 2)
        flat = xp.reshape(C, T, 441)
        xt_all.append(flat[:, 0:16].reshape(C, 16 * 441).astype(
            ml_dtypes.bfloat16))
        xt2_all.append(flat[:, 16:32].reshape(C, 16 * 441).astype(
            ml_dtypes.bfloat16))
    for core in range(NCORE):
        b, hb = core // 2, (core % 2) * HPC
        # stage-A weights: block-diagonal per tap; rows 0-63 (half-0 img)
        # feed cols 0-31, rows 64-127 (half-1 img) feed cols 32-63
        wk = np.zeros((128, 9 * 2 * M32), np.float32)
        for tap in range(9):
            dy, dx = tap // 3, tap % 3
            for h in range(HPC):
                wk[0:64, 64 * tap + D * h : 64 * tap + D * (h + 1)] = \
                    w_kv[hb + h, dy, dx]
                wk[64:128, 64 * tap + 32 + D * h : 64 * tap + 32 + D * (h + 1)] = \
                    w_kv[hb + h, dy, dx]
        # stage-B weights: block (q, tap) maps img j of group (row 32j +
        # 16h + d) to score column 8q + 2j + h
        ws = np.zeros((128, 4, 9, M32), np.float32)
        for qq in range(4):
            for tap in range(9):
                dy, dx = tap // 3, tap % 3
                for j in range(4):
                    for h in range(HPC):
                        ws[32 * j + D * h : 32 * j + D * (h + 1), qq, tap,
                           8 * qq + 2 * j + h] = w_att[hb + h, dy, dx, :D, 0]
        ws = ws.reshape(128, 4 * 9 * M32)
        bv = np.zeros((128, 1), np.float32)
        bv[:, 0] = np.tile(np.concatenate([b_kv[hb], b_kv[hb + 1]]), 4)
        in_maps.append({"x_t": xt_all[b], "x_t2": xt2_all[b],
                        "w_kv_t": wk.astype(ml_dtypes.bfloat16),
                        "w_s_t": ws.astype(ml_dtypes.bfloat16),
                        "b_vec": bv, "ident": ident})
    return in_maps


def assemble(results):
    out = np.empty((B, T, HS, WS, C), np.float32)
    for core in range(NCORE):
        b, hb = core // 2, (core % 2) * M32
        yc = np.asarray(results[core]["y"]).reshape(HS, WS, M32)
        out[b, :, :, :, hb : hb + M32] = yc[None]
    return out


def kernel(x, w_q, b_q, w_kv, b_kv, w_att, b_att, **_unused):
    nc = _build_program()
    in_maps = make_in_maps(x, w_kv, b_kv, w_att)
    res = run_bass_kernel_spmd(nc, in_maps, core_ids=list(range(NCORE)))
    return assemble(res.results)


if __name__ == "__main__":
    rng = np.random.default_rng(0)
    ins = {
        "x": rng.standard_normal((B, T, HS, WS, C)).astype(np.float32),
        "w_q": rng.standard_normal((NH, 3, 3, C, D)).astype(np.float32) * 0.05,
        "b_q": np.zeros((NH, D), np.float32),
        "w_kv": rng.standard_normal((NH, 3, 3, C, D)).astype(np.float32) * 0.05,
        "b_kv": np.zeros((NH, D), np.float32),
        "w_att": rng.standard_normal((NH, 3, 3, 2 * D, 1)).astype(np.float32) * 0.05,
        "b_att": np.zeros((NH, 1), np.float32),
    }
    out = kernel(**ins)
    print("kernel output", out.shape, out.dtype)
